# revision 35
# baseline (speedup 1.0000x reference)
"""CQT layer kernel for Trainium2 (8 NeuronCores, SPMD) — sparse band version.

The CQT filterbank is ~82% zeros: bin k's filter has a centered support of
Nk ~ 63864 * 2^(-k/66) samples.  We exploit this at 16-bin granularity:
group g = bins [16g, 16g+16) x {re, im} = 32 channels whose joint support
spans chunks [c0(g), c1(g)) of the 128-sample contraction grid (hop == 128,
so audio reshaped [128, cols] makes the strided conv a chunked matmul).

Each (group, chunk) unit is a [128k x 32ch] stationary matmul against 348
moving columns (174 frames x 2 batch, (t,b)-interleaved), run in 128x32
column-tiled PE mode: 4 tiles (lanes) sustain ~4 matmuls per 348 cycles.

SPMD uniformity: all 8 cores run the IDENTICAL program.
 - Regular groups (n > 16 chunks): core i takes chunks c0(g)+i, +8, ...
   (stride 8).  The program addresses x at chunk c0(g)+8t; the per-core
   "+i" shift is folded into the DATA by shifting core i's x buffer left
   by i chunks.  Padded to T(g) = ceil(n/8) slots with zero weights.
 - Small groups (n <= 16): assigned WHOLE to one core as fixed-length
   runs (R slots each, same template on every core).  Each run's x
   window lives in a per-core scratch region of the x buffer, so tiny
   groups cost 1-2 evictions per core instead of one per group.

Per core: ~424 units => ~16 us PE, ~3.5 MB weights (fully preloaded into
SBUF), fp16 partials drained by a handful of chunked DMAs.  Host sums the
per-core 32-row partials per group, then magnitude + power_to_db with an
exact fp64 recompute of near-silent bins.

Self-contained: only needs numpy + the concourse toolchain at /opt/trn_rl_repo.
"""
import os
import sys

sys.path.insert(0, "/opt/trn_rl_repo")
import numpy as np

# ---- problem constants (hardcoded from the CQT layer spec) ----
B = 2
AUDIO_LEN = 22016
N_BINS = 528
NCH = 2 * N_BINS          # 1056 conv channels (re, im)
HOP = 128
FRAMES = 173
AMIN = 1e-10
TOP_DB = 80.0

K = 128                   # PE contraction tile == HOP
NT = 174                  # frames padded to even
NF = B * NT               # 348 moving columns per matmul
GB = 16                   # bins per group
NG = N_BINS // GB         # 33 groups
CH = 2 * GB               # 32 channels per group (re+im)
N_CORES = 8
NCHUNK = 499              # ceil(L / 128) for L in (63744, 63872]
XCOLS = 688               # main x region chunk-columns (>= 499+7+174)
R_RUN = 10                # slots per small-group run
SMALL_MAX = 16            # groups with <= this many chunks are run-assigned
SCRW = R_RUN + NT + 2     # scratch region width per run (col-pairs)
GROUP = int(os.environ.get("CQT_GROUP", "128"))  # weight slots per DMA group
WARM_ROUNDS = int(os.environ.get("CQT_WARM", "10"))

DB_ERR_TARGET = 0.02      # refine bins whose worst-case dB error exceeds this
CONV_EPS = 1.5e-3         # fp16 matmul + fp16 partial-eviction error vs conv rms

_prog_cache = {}


def _schedule(items):
    """LPT lane assignment + round-robin issue order over generic items.

    items: list of (kind, ident, length).  Returns (issue, islots, lanes)
    where issue[s] = (lane, item_index, t), islots[i] = slot list of item i,
    lanes[L] = ordered item-index list (the lane's eviction order).
    """
    order = sorted(range(len(items)), key=lambda i: -items[i][2])
    lanes = [[] for _ in range(4)]
    loads = [0] * 4
    for i in order:
        L = loads.index(min(loads))
        lanes[L].append(i)
        loads[L] += items[i][2]
    for L in range(4):
        # biggest item first (ramps the PE p-state on a long gapless run),
        # small ones mid-stream, second-biggest last for a quiet drain
        ls = sorted(lanes[L], key=lambda i: -items[i][2])
        lanes[L] = ls[:1] + ls[:0:-1]
    ptr = [0] * 4
    tcur = [0] * 4
    issue = []
    islots = [[None] * items[i][2] for i in range(len(items))]
    done, total = 0, sum(it[2] for it in items)
    while done < total:
        for L in range(4):
            if ptr[L] >= len(lanes[L]):
                continue
            i = lanes[L][ptr[L]]
            t = tcur[L]
            islots[i][t] = len(issue)
            issue.append((L, i, t))
            done += 1
            tcur[L] += 1
            if tcur[L] >= items[i][2]:
                ptr[L] += 1
                tcur[L] = 0
    return issue, islots, lanes


def _build_program(items, issue, lanes, c0, xt_cols):
    from concourse import bacc, mybir
    from concourse.tile import TileContext

    dt = mybir.dt
    S = len(issue)

    nc = bacc.Bacc(None, target_bir_lowering=False)
    xt_p = nc.declare_dram_parameter("xt", [K, B * xt_cols], dt.float16, isOutput=False)
    wm_p = nc.declare_dram_parameter("wm", [K, S * CH], dt.float16, isOutput=False)
    om_p = [
        nc.declare_dram_parameter(f"om{L}", [CH, max(len(lanes[L]), 1) * NF],
                                  dt.float16, isOutput=True)
        for L in range(4)
    ]

    # weight DMA groups: small first so the PE starts streaming early;
    # everything is preloaded (wpool holds all groups), so no WAR stalls
    groups = []
    k0 = 0
    for gsz in (8, 16, 32):
        groups.append((k0, gsz))
        k0 += gsz
    while k0 < S:
        cnt = min(GROUP, S - k0)
        groups.append((k0, cnt))
        k0 += cnt

    with TileContext(nc) as tc:
        with (
            tc.tile_pool(name="stat", bufs=1) as stat,
            tc.tile_pool(name="wpool", bufs=len(groups)) as wpool,
            tc.tile_pool(name="ps", bufs=1, space="PSUM") as ps,
        ):
            # disjoint per-lane PSUM bank pairs: lane L double-buffers in
            # banks {2L, 2L+1}; no cross-lane bank conflicts with evictions
            pst = [ps.tile([K, NF], dt.float32, tag=f"ps{j}", name=f"ps{j}")
                   for j in range(8)]

            # PE warm-up in 128x32 col-tiled mode on a memset tile (no DMA
            # dependency): ramps the p-state while the first weights land
            warm_sb = stat.tile([K, NF], dt.float16)
            nc.gpsimd.memset(warm_sb[:], 0.0)
            for r in range(WARM_ROUNDS):
                for L in range(4):
                    nc.tensor.matmul(
                        pst[(2 * L + r) % 8][32 * L:32 * (L + 1), :],
                        warm_sb[:, :CH], warm_sb[:],
                        start=True, stop=True, tile_position=(0, 32 * L),
                    )

            # xt via the Activation-engine DGE so the Sync queue starts on
            # weight groups immediately
            xt_sb = stat.tile([K, B * xt_cols], dt.float16)
            nc.scalar.dma_start(xt_sb[:], xt_p[:])
            x3 = xt_sb[:].rearrange("p (c b) -> p c b", b=B)

            # persistent fp16 eviction buffer: lane L's e-th finished item
            # lands at rows [32L, 32L+32), cols [e*NF, (e+1)*NF); drained by
            # two chunked DMAs per lane (mid-stream + end)
            om_sb = stat.tile([K, max(len(l) for l in lanes) * NF], dt.float16)

            wgs = []
            for (g0, cnt) in groups:
                wg = wpool.tile([K, GROUP * CH], dt.float16, tag="wg")
                nc.sync.dma_start(
                    wg[:, :cnt * CH],
                    wm_p[:, g0 * CH:(g0 + cnt) * CH],
                )
                wgs.append(wg)

            # main stream: issue-ordered col-tiled matmuls, 4 lanes
            pcur = [0] * 4       # psum buffer (within the lane's pair)
            ev_n = [0] * 4       # finished items per lane
            evict_total = 0
            gi = 0               # current weight DMA group
            for s, (L, i, t) in enumerate(issue):
                g0, cnt = groups[gi]
                if s >= g0 + cnt:
                    gi += 1
                    g0, cnt = groups[gi]
                wg = wgs[gi]
                j = s - g0
                kind, ident, tlen = items[i]
                ptile = pst[2 * L + pcur[L]]
                psl = ptile[32 * L:32 * (L + 1), :]
                if kind == "g":
                    cprog = c0[ident] + 8 * t
                else:  # small-group run: fixed scratch region
                    cprog = XCOLS + ident * SCRW + t
                nc.tensor.matmul(
                    psl,
                    wg[:, j * CH:(j + 1) * CH],
                    x3[:, cprog:cprog + NT, :],
                    start=(t == 0),
                    stop=(t == tlen - 1),
                    tile_position=(0, 32 * L),
                )
                if t == tlen - 1:
                    e = ev_n[L]
                    rows = slice(32 * L, 32 * (L + 1))
                    dst = om_sb[rows, e * NF:(e + 1) * NF]
                    if evict_total % 2 == 0:
                        nc.vector.tensor_copy(dst, psl)
                    else:
                        nc.scalar.activation(
                            dst, psl, mybir.ActivationFunctionType.Copy)
                    ev_n[L] += 1
                    evict_total += 1
                    pcur[L] ^= 1
                    half = len(lanes[L]) // 2
                    if ev_n[L] == half:
                        nc.sync.dma_start(
                            om_p[L][:, :half * NF],
                            om_sb[rows, :half * NF],
                        )

            # drain: per-lane second-half out DMA, spread across engine DGEs
            drain_eng = [nc.sync, nc.scalar, nc.gpsimd, nc.sync]
            for L in range(4):
                half = len(lanes[L]) // 2
                full = len(lanes[L])
                if full > half:
                    drain_eng[L].dma_start(
                        om_p[L][:, half * NF:full * NF],
                        om_sb[32 * L:32 * (L + 1), half * NF:full * NF],
                    )

    nc.finalize()
    return nc


LAST_RESULTS = None


def kernel(y, kern_r, kern_i):
    global LAST_RESULTS
    from concourse.bass_utils import run_bass_kernel_spmd

    y = np.asarray(y, dtype=np.float32)
    kern_r = np.asarray(kern_r, dtype=np.float32)
    kern_i = np.asarray(kern_i, dtype=np.float32)

    L_in = kern_r.shape[1]
    pad = L_in // 2
    W = np.concatenate([kern_r, kern_i], axis=0)           # [1056, L]
    LPAD = NCHUNK * K
    assert L_in <= LPAD, L_in
    Wp = np.zeros((NCH, LPAD), np.float32)
    Wp[:, :L_in] = W
    W3 = Wp.reshape(NCH, NCHUNK, K)                        # [ch, chunk, 128]

    # ---- per-group chunk support from the actual kernel arrays ----
    nz = np.abs(W) > 0
    any_nz = nz.any(axis=1)
    lo_k = np.where(any_nz, nz.argmax(axis=1), 0)
    hi_k = np.where(any_nz, L_in - nz[:, ::-1].argmax(axis=1), 1)
    chs = [np.r_[GB * g:GB * (g + 1), N_BINS + GB * g:N_BINS + GB * (g + 1)]
           for g in range(NG)]
    c0, n = [], []
    for g in range(NG):
        lo = int(lo_k[chs[g]].min())
        hi = int(hi_k[chs[g]].max())
        a = lo // K
        b = min(-(-hi // K), NCHUNK)
        c0.append(a)
        n.append(max(b - a, 1))

    # regular groups stride-8 across cores; small groups become whole-core
    # pieces packed into fixed runs
    items = []            # (kind, ident, length)
    for g in range(NG):
        if n[g] > SMALL_MAX:
            items.append(("g", g, -(-n[g] // N_CORES)))
    pieces = []           # (g, chunk offset, len) in run-slot order
    for g in range(NG):
        if n[g] <= SMALL_MAX:
            off = 0
            while off < n[g]:
                ln = min(R_RUN, n[g] - off)
                pieces.append((g, off, ln))
                off += ln
    n_runs = -(-len(pieces) // N_CORES) if pieces else 0
    for r in range(n_runs):
        items.append(("r", r, R_RUN))

    issue, islots, lanes = _schedule(items)
    S = len(issue)
    xt_cols = XCOLS + n_runs * SCRW

    # ---- host prep: per-core x buffers (global shift + run scratch) ----
    XF = XCOLS + N_CORES
    x_pad = np.zeros((B, XF * K), np.float32)
    x_pad[:, pad:pad + AUDIO_LEN] = y
    xT = x_pad.reshape(B, XF, K).transpose(0, 2, 1)        # [B, 128, XF]
    xt16 = np.ascontiguousarray(xT).astype(np.float16)

    item_of = {("r", r): i for i, (kind, r, _) in enumerate(items)
               if kind == "r"}
    in_maps = []
    for i in range(N_CORES):
        xt_i = np.zeros((K, B * xt_cols), np.float16)
        xt_i[:, 0:2 * XCOLS:2] = xt16[0, :, i:i + XCOLS]
        xt_i[:, 1:2 * XCOLS:2] = xt16[1, :, i:i + XCOLS]
        wm = np.zeros((S, CH, K), np.float32)
        for g in range(NG):
            if n[g] <= SMALL_MAX:
                continue
            tmax = -(-(n[g] - i) // N_CORES) if n[g] > i else 0
            if tmax <= 0:
                continue
            cs = c0[g] + i + N_CORES * np.arange(tmax)
            it = next(ii for ii, (kind, ident, _) in enumerate(items)
                      if kind == "g" and ident == g)
            sl = np.asarray(islots[it][:tmax])
            wm[sl] = W3[chs[g]][:, cs, :].transpose(1, 0, 2)
        for r in range(n_runs):
            p = r * N_CORES + i
            if p >= len(pieces):
                continue
            g, off, ln = pieces[p]
            ca = c0[g] + off
            # scratch x: window [ca, ca + ln-1 + NT)
            scr = XCOLS + r * SCRW
            wlen = min(ln - 1 + NT, XF - ca)
            xt_i[:, 2 * scr:2 * (scr + wlen):2] = xt16[0, :, ca:ca + wlen]
            xt_i[:, 2 * scr + 1:2 * (scr + wlen) + 1:2] = xt16[1, :, ca:ca + wlen]
            it = item_of[("r", r)]
            sl = np.asarray(islots[it][:ln])
            wm[sl] = W3[chs[g]][:, ca:ca + ln, :].transpose(1, 0, 2)
        wm_host = np.ascontiguousarray(
            wm.transpose(2, 0, 1).reshape(K, S * CH)).astype(np.float16)
        in_maps.append({"xt": np.ascontiguousarray(xt_i), "wm": wm_host})

    key = (tuple(c0), tuple(n))
    if key not in _prog_cache:
        _prog_cache[key] = _build_program(items, issue, lanes, c0, xt_cols)
    nc = _prog_cache[key]

    LAST_RESULTS = run_bass_kernel_spmd(
        nc, in_maps, list(range(N_CORES)),
        trace=bool(os.environ.get("CQT_TRACE")),
    )
    results = LAST_RESULTS.results

    # ---- host post: assemble conv from per-core per-lane partials ----
    conv = np.zeros((NCH, B, NT), np.float64)
    for i in range(N_CORES):
        for L in range(4):
            om = results[i][f"om{L}"].astype(np.float64)
            om = om.reshape(CH, -1, NT, B)                 # cols = (e, t, b)
            for e, it in enumerate(lanes[L]):
                kind, ident, _ = items[it]
                if kind == "g":
                    conv[chs[ident]] += om[:, e].transpose(0, 2, 1)
                else:
                    p = ident * N_CORES + i
                    if p < len(pieces):
                        conv[chs[pieces[p][0]]] += om[:, e].transpose(0, 2, 1)
    conv = conv[:, :, :FRAMES]

    re = conv[:N_BINS]
    im = conv[N_BINS:]
    mag = np.sqrt(re * re + im * im)                       # [528, B, 173]

    # ---- host refinement: exact recompute of near-silent bins ----
    conv_rms = float(np.sqrt(np.mean(mag * mag)))
    thresh = 4.343 * CONV_EPS * conv_rms / DB_ERR_TARGET
    fix = np.argwhere(mag < thresh)                        # rows: (bin, b, t)
    if len(fix):
        W64 = W.astype(np.float64)
        xp64 = np.zeros((B, 2 * pad + AUDIO_LEN), np.float64)
        xp64[:, pad:pad + AUDIO_LEN] = y
        for b in range(B):
            sel = fix[fix[:, 1] == b]
            if not len(sel):
                continue
            for t in np.unique(sel[:, 2]):
                bins = sel[sel[:, 2] == t][:, 0]
                win = xp64[b, t * HOP:t * HOP + L_in]
                re[bins, b, t] = W64[bins] @ win
                im[bins, b, t] = W64[bins + N_BINS] @ win
        mag = np.sqrt(re * re + im * im)

    ref = max(mag.max(), AMIN)
    log_spec = 10.0 * np.log10(np.maximum(mag, AMIN)) - 10.0 * np.log10(ref)
    log_spec = np.maximum(log_spec, log_spec.max() - TOP_DB)
    return np.ascontiguousarray(log_spec.transpose(1, 2, 0)).astype(np.float32)


# revision 41
# speedup vs baseline: 1.1886x; 1.1886x over previous
"""CQT layer kernel for Trainium2 (8 NeuronCores, SPMD) — sparse band version.

The CQT filterbank is ~82% zeros: bin k's filter has a centered support of
Nk ~ 63864 * 2^(-k/66) samples.  We exploit this at 16-bin granularity:
group g = bins [16g, 16g+16) x {re, im} = 32 channels whose joint support
spans chunks [c0(g), c1(g)) of the 128-sample contraction grid (hop == 128,
so audio reshaped [128, cols] makes the strided conv a chunked matmul).

Each (group, chunk) unit is a [128k x 32ch] stationary matmul against 348
moving columns (174 frames x 2 batch, (t,b)-interleaved), run in 128x32
column-tiled PE mode: 4 tiles (lanes) sustain ~4 matmuls per 348 cycles.

SPMD uniformity: all 8 cores run the IDENTICAL program.
 - Regular groups (n > 16 chunks): core i takes chunks c0(g)+i, +8, ...
   (stride 8).  The program addresses x at chunk c0(g)+8t; the per-core
   "+i" shift is folded into the DATA by shifting core i's x buffer left
   by i chunks.  Padded to T(g) = ceil(n/8) slots with zero weights.
 - Small groups (n <= 16): assigned WHOLE to one core as fixed-length
   runs (R slots each, same template on every core).  Each run's x
   window lives in a per-core scratch region of the x buffer, so tiny
   groups cost 1-2 evictions per core instead of one per group.

Per core: ~424 units => ~16 us PE, ~3.5 MB weights (fully preloaded into
SBUF), fp16 partials drained by a handful of chunked DMAs.  Host sums the
per-core 32-row partials per group, then magnitude + power_to_db with an
exact fp64 recompute of near-silent bins.

Self-contained: only needs numpy + the concourse toolchain at /opt/trn_rl_repo.
"""
import os
import sys

sys.path.insert(0, "/opt/trn_rl_repo")
import numpy as np

# ---- problem constants (hardcoded from the CQT layer spec) ----
B = 2
AUDIO_LEN = 22016
N_BINS = 528
NCH = 2 * N_BINS          # 1056 conv channels (re, im)
HOP = 128
FRAMES = 173
AMIN = 1e-10
TOP_DB = 80.0

K = 128                   # PE contraction tile == HOP
NT = 174                  # frames padded to even
NF = B * NT               # 348 moving columns per matmul
GB = 16                   # bins per group
NG = N_BINS // GB         # 33 groups
CH = 2 * GB               # 32 channels per group (re+im)
N_CORES = 8
NCHUNK = 499              # ceil(L / 128) for L in (63744, 63872]
XCOLS = 688               # main x region chunk-columns (>= 499+7+174)
R_RUN = 10                # slots per small-group run
SMALL_MAX = 16            # groups with <= this many chunks are run-assigned
SCRW = R_RUN + NT + 2     # scratch region width per run (col-pairs)
GROUP = int(os.environ.get("CQT_GROUP", "128"))  # weight slots per DMA group
WARM_ROUNDS = int(os.environ.get("CQT_WARM", "10"))

DB_ERR_TARGET = 0.02      # refine bins whose worst-case dB error exceeds this
CONV_EPS = 1.5e-3         # fp16 matmul + fp16 partial-eviction error vs conv rms

_prog_cache = {}


def _schedule(items):
    """LPT lane assignment + round-robin issue order over generic items.

    items: list of (kind, ident, length).  Returns (issue, islots, lanes)
    where issue[s] = (lane, item_index, t), islots[i] = slot list of item i,
    lanes[L] = ordered item-index list (the lane's eviction order).
    """
    order = sorted(range(len(items)), key=lambda i: -items[i][2])
    lanes = [[] for _ in range(4)]
    loads = [0] * 4
    for i in order:
        L = loads.index(min(loads))
        lanes[L].append(i)
        loads[L] += items[i][2]
    for L in range(4):
        # biggest item first (ramps the PE p-state on a long gapless run),
        # small ones mid-stream, second-biggest last for a quiet drain
        ls = sorted(lanes[L], key=lambda i: -items[i][2])
        lanes[L] = ls[:1] + ls[:0:-1]
    ptr = [0] * 4
    tcur = [0] * 4
    issue = []
    islots = [[None] * items[i][2] for i in range(len(items))]
    done, total = 0, sum(it[2] for it in items)
    while done < total:
        for L in range(4):
            if ptr[L] >= len(lanes[L]):
                continue
            i = lanes[L][ptr[L]]
            t = tcur[L]
            islots[i][t] = len(issue)
            issue.append((L, i, t))
            done += 1
            tcur[L] += 1
            if tcur[L] >= items[i][2]:
                ptr[L] += 1
                tcur[L] = 0
    return issue, islots, lanes


def _build_program(items, issue, lanes, c0, xt_cols):
    from concourse import bacc, mybir
    from concourse.tile import TileContext

    dt = mybir.dt
    S = len(issue)

    nc = bacc.Bacc(None, target_bir_lowering=False)
    xt_p = nc.declare_dram_parameter("xt", [K, B * xt_cols], dt.float16, isOutput=False)
    wm_p = nc.declare_dram_parameter("wm", [K, S * CH], dt.float16, isOutput=False)
    om_p = [
        nc.declare_dram_parameter(f"om{L}", [CH, max(len(lanes[L]), 1) * NF],
                                  dt.float16, isOutput=True)
        for L in range(4)
    ]

    # weight DMA groups: small first so the PE starts streaming early;
    # everything is preloaded (wpool holds all groups), so no WAR stalls
    groups = []
    k0 = 0
    for gsz in (8, 16, 32):
        groups.append((k0, gsz))
        k0 += gsz
    while k0 < S:
        cnt = min(GROUP, S - k0)
        groups.append((k0, cnt))
        k0 += cnt

    with TileContext(nc) as tc:
        with (
            tc.tile_pool(name="stat", bufs=1) as stat,
            tc.tile_pool(name="wpool", bufs=len(groups)) as wpool,
            tc.tile_pool(name="ps", bufs=1, space="PSUM") as ps,
        ):
            # disjoint per-lane PSUM bank pairs: lane L double-buffers in
            # banks {2L, 2L+1}; no cross-lane bank conflicts with evictions
            pst = [ps.tile([K, NF], dt.float32, tag=f"ps{j}", name=f"ps{j}")
                   for j in range(8)]

            # PE warm-up in 128x32 col-tiled mode on a memset tile (no DMA
            # dependency): ramps the p-state while the first weights land
            warm_sb = stat.tile([K, NF], dt.float16)
            nc.gpsimd.memset(warm_sb[:], 0.0)
            for r in range(WARM_ROUNDS):
                for L in range(4):
                    nc.tensor.matmul(
                        pst[(2 * L + r) % 8][32 * L:32 * (L + 1), :],
                        warm_sb[:, :CH], warm_sb[:],
                        start=True, stop=True, tile_position=(0, 32 * L),
                    )

            # xt via the Activation-engine DGE so the Sync queue starts on
            # weight groups immediately
            xt_sb = stat.tile([K, B * xt_cols], dt.float16)
            nc.scalar.dma_start(xt_sb[:], xt_p[:])
            x3 = xt_sb[:].rearrange("p (c b) -> p c b", b=B)

            # persistent fp16 eviction buffer: lane L's e-th finished item
            # lands at rows [32L, 32L+32), cols [e*NF, (e+1)*NF); drained by
            # two chunked DMAs per lane (mid-stream + end)
            om_sb = stat.tile([K, max(len(l) for l in lanes) * NF], dt.float16)

            wgs = []
            for (g0, cnt) in groups:
                wg = wpool.tile([K, GROUP * CH], dt.float16, tag="wg")
                nc.sync.dma_start(
                    wg[:, :cnt * CH],
                    wm_p[:, g0 * CH:(g0 + cnt) * CH],
                )
                wgs.append(wg)

            # main stream: issue-ordered col-tiled matmuls, 4 lanes
            pcur = [0] * 4       # psum buffer (within the lane's pair)
            ev_n = [0] * 4       # finished items per lane
            evict_total = 0
            gi = 0               # current weight DMA group
            for s, (L, i, t) in enumerate(issue):
                g0, cnt = groups[gi]
                if s >= g0 + cnt:
                    gi += 1
                    g0, cnt = groups[gi]
                wg = wgs[gi]
                j = s - g0
                kind, ident, tlen = items[i]
                ptile = pst[2 * L + pcur[L]]
                psl = ptile[32 * L:32 * (L + 1), :]
                if kind == "g":
                    cprog = c0[ident] + 8 * t
                else:  # small-group run: fixed scratch region
                    cprog = XCOLS + ident * SCRW + t
                nc.tensor.matmul(
                    psl,
                    wg[:, j * CH:(j + 1) * CH],
                    x3[:, cprog:cprog + NT, :],
                    start=(t == 0),
                    stop=(t == tlen - 1),
                    tile_position=(0, 32 * L),
                )
                if t == tlen - 1:
                    e = ev_n[L]
                    rows = slice(32 * L, 32 * (L + 1))
                    dst = om_sb[rows, e * NF:(e + 1) * NF]
                    if evict_total % 2 == 0:
                        nc.vector.tensor_copy(dst, psl)
                    else:
                        nc.scalar.activation(
                            dst, psl, mybir.ActivationFunctionType.Copy)
                    ev_n[L] += 1
                    evict_total += 1
                    pcur[L] ^= 1
                    half = len(lanes[L]) // 2
                    if ev_n[L] == half:
                        nc.sync.dma_start(
                            om_p[L][:, :half * NF],
                            om_sb[rows, :half * NF],
                        )

            # drain: per-lane second-half out DMA, spread across engine DGEs
            drain_eng = [nc.sync, nc.scalar, nc.gpsimd, nc.sync]
            for L in range(4):
                half = len(lanes[L]) // 2
                full = len(lanes[L])
                if full > half:
                    drain_eng[L].dma_start(
                        om_p[L][:, half * NF:full * NF],
                        om_sb[32 * L:32 * (L + 1), half * NF:full * NF],
                    )

    nc.finalize()
    return nc


LAST_RESULTS = None


def kernel(y, kern_r, kern_i):
    global LAST_RESULTS
    from concourse.bass_utils import run_bass_kernel_spmd

    y = np.asarray(y, dtype=np.float32)
    kern_r = np.asarray(kern_r, dtype=np.float32)
    kern_i = np.asarray(kern_i, dtype=np.float32)

    L_in = kern_r.shape[1]
    pad = L_in // 2
    W = np.concatenate([kern_r, kern_i], axis=0)           # [1056, L]
    LPAD = NCHUNK * K
    assert L_in <= LPAD, L_in
    Wp = np.zeros((NCH, LPAD), np.float32)
    Wp[:, :L_in] = W
    W3 = Wp.reshape(NCH, NCHUNK, K)                        # [ch, chunk, 128]

    # ---- per-group chunk support from the actual kernel arrays ----
    nz = np.abs(W) > 0
    any_nz = nz.any(axis=1)
    lo_k = np.where(any_nz, nz.argmax(axis=1), 0)
    hi_k = np.where(any_nz, L_in - nz[:, ::-1].argmax(axis=1), 1)
    chs = [np.r_[GB * g:GB * (g + 1), N_BINS + GB * g:N_BINS + GB * (g + 1)]
           for g in range(NG)]
    c0, n = [], []
    for g in range(NG):
        lo = int(lo_k[chs[g]].min())
        hi = int(hi_k[chs[g]].max())
        a = lo // K
        b = min(-(-hi // K), NCHUNK)
        c0.append(a)
        n.append(max(b - a, 1))

    # regular groups stride-8 across cores; small groups become whole-core
    # pieces packed into fixed runs
    items = []            # (kind, ident, length)
    for g in range(NG):
        if n[g] > SMALL_MAX:
            items.append(("g", g, -(-n[g] // N_CORES)))
    pieces = []           # (g, chunk offset, len) in run-slot order
    for g in range(NG):
        if n[g] <= SMALL_MAX:
            off = 0
            while off < n[g]:
                ln = min(R_RUN, n[g] - off)
                pieces.append((g, off, ln))
                off += ln
    n_runs = -(-len(pieces) // N_CORES) if pieces else 0
    for r in range(n_runs):
        items.append(("r", r, R_RUN))

    issue, islots, lanes = _schedule(items)
    S = len(issue)
    xt_cols = XCOLS + n_runs * SCRW

    # ---- host prep: per-core x buffers (global shift + run scratch) ----
    XF = XCOLS + N_CORES
    x_pad = np.zeros((B, XF * K), np.float32)
    x_pad[:, pad:pad + AUDIO_LEN] = y
    xT = x_pad.reshape(B, XF, K).transpose(0, 2, 1)        # [B, 128, XF]
    xt16 = np.ascontiguousarray(xT).astype(np.float16)

    item_of = {("r", r): i for i, (kind, r, _) in enumerate(items)
               if kind == "r"}
    in_maps = []
    for i in range(N_CORES):
        xt_i = np.zeros((K, B * xt_cols), np.float16)
        xt_i[:, 0:2 * XCOLS:2] = xt16[0, :, i:i + XCOLS]
        xt_i[:, 1:2 * XCOLS:2] = xt16[1, :, i:i + XCOLS]
        wm = np.zeros((S, CH, K), np.float32)
        for g in range(NG):
            if n[g] <= SMALL_MAX:
                continue
            tmax = -(-(n[g] - i) // N_CORES) if n[g] > i else 0
            if tmax <= 0:
                continue
            cs = c0[g] + i + N_CORES * np.arange(tmax)
            it = next(ii for ii, (kind, ident, _) in enumerate(items)
                      if kind == "g" and ident == g)
            sl = np.asarray(islots[it][:tmax])
            wm[sl] = W3[chs[g]][:, cs, :].transpose(1, 0, 2)
        for r in range(n_runs):
            p = r * N_CORES + i
            if p >= len(pieces):
                continue
            g, off, ln = pieces[p]
            ca = c0[g] + off
            # scratch x: window [ca, ca + ln-1 + NT)
            scr = XCOLS + r * SCRW
            wlen = min(ln - 1 + NT, XF - ca)
            xt_i[:, 2 * scr:2 * (scr + wlen):2] = xt16[0, :, ca:ca + wlen]
            xt_i[:, 2 * scr + 1:2 * (scr + wlen) + 1:2] = xt16[1, :, ca:ca + wlen]
            it = item_of[("r", r)]
            sl = np.asarray(islots[it][:ln])
            wm[sl] = W3[chs[g]][:, ca:ca + ln, :].transpose(1, 0, 2)
        wm_host = np.ascontiguousarray(
            wm.transpose(2, 0, 1).reshape(K, S * CH)).astype(np.float16)
        in_maps.append({"xt": np.ascontiguousarray(xt_i), "wm": wm_host})

    key = (tuple(c0), tuple(n))
    if key not in _prog_cache:
        _prog_cache[key] = _build_program(items, issue, lanes, c0, xt_cols)
    nc = _prog_cache[key]

    LAST_RESULTS = run_bass_kernel_spmd(
        nc, in_maps, list(range(N_CORES)),
        trace=bool(os.environ.get("CQT_TRACE")),
    )
    results = LAST_RESULTS.results

    # ---- host post: assemble conv from per-core per-lane partials ----
    conv = np.zeros((NCH, B, NT), np.float64)
    for i in range(N_CORES):
        for L in range(4):
            om = results[i][f"om{L}"].astype(np.float64)
            om = om.reshape(CH, -1, NT, B)                 # cols = (e, t, b)
            for e, it in enumerate(lanes[L]):
                kind, ident, _ = items[it]
                if kind == "g":
                    conv[chs[ident]] += om[:, e].transpose(0, 2, 1)
                else:
                    p = ident * N_CORES + i
                    if p < len(pieces):
                        conv[chs[pieces[p][0]]] += om[:, e].transpose(0, 2, 1)
    conv = conv[:, :, :FRAMES]

    re = conv[:N_BINS]
    im = conv[N_BINS:]
    mag = np.sqrt(re * re + im * im)                       # [528, B, 173]

    # ---- host refinement: exact recompute of near-silent bins ----
    conv_rms = float(np.sqrt(np.mean(mag * mag)))
    thresh = 4.343 * CONV_EPS * conv_rms / DB_ERR_TARGET
    fix = np.argwhere(mag < thresh)                        # rows: (bin, b, t)
    if len(fix):
        W64 = W.astype(np.float64)
        xp64 = np.zeros((B, 2 * pad + AUDIO_LEN), np.float64)
        xp64[:, pad:pad + AUDIO_LEN] = y
        for b in range(B):
            sel = fix[fix[:, 1] == b]
            if not len(sel):
                continue
            for t in np.unique(sel[:, 2]):
                bins = sel[sel[:, 2] == t][:, 0]
                win = xp64[b, t * HOP:t * HOP + L_in]
                re[bins, b, t] = W64[bins] @ win
                im[bins, b, t] = W64[bins + N_BINS] @ win
        mag = np.sqrt(re * re + im * im)

    ref = max(mag.max(), AMIN)
    log_spec = 10.0 * np.log10(np.maximum(mag, AMIN)) - 10.0 * np.log10(ref)
    log_spec = np.maximum(log_spec, log_spec.max() - TOP_DB)
    return np.ascontiguousarray(log_spec.transpose(1, 2, 0)).astype(np.float32)
